# revision 20
# baseline (speedup 1.0000x reference)
"""DGCN (diffusion graph conv) Trainium2 Bass kernel.

Reference computation (per batch b, time t):
    h   = relu(st_emb @ W1 + b1)              # [T,1,32]
    lam = 1 + relu(h @ W2 + b2)               # [T,1,1]
    c1  = 2 - 2/lam ;  c2 = 2/lam             # scalars per t
    out[b,t] = c1[t] * (x[b,t] @ W0) + c2[t] * ((adj @ x[b,t]) @ W1g) + bias
where weights = [W0; W1g] with shape [2, 64, 64].

Strategy: data-parallel over batch B=8 across the 8 NeuronCores (adj and
weights replicated).  Per core, with x_b node-major X' [N, T*C] and adjT:
    YT[tc, i] = sum_j X'[j, tc] * adjT[j, i]          (one big fp16 matmul,
                                                       K=2048, M=768, N=2048)
    out pairs: for each pair of timesteps (2m, 2m+1), block-diagonal
    stationaries diag(c*Wk, c*Wk) [128,128] contract the 128-row
    (two-timestep) channel-major slabs of x and YT, accumulating the
    identity-term and adjacency-term into one PSUM bank.

All heavy tensors move and multiply in fp16 (11-bit mantissa ~ fp32r/TF32
precision, half the HBM traffic of the fp32r variant).  The channel-major
copy of x is prepared host-side (pure layout transform, like the adj
transpose), which removes all on-device PE transposes.  The tiny lambda-MLP
runs on device in full fp32.

Measured HW notes (axon-tunneled trn2, 2026-08-08): the real PE streams
fp16 matmuls at ~2 rows/cycle -- twice the cost-model rate -- so the fp16
switch halved PE time in addition to halving DMA.  An isolated PE-only
variant of this instruction mix ran ~43-48 us/iter and a DMA-only variant
~10 us/iter: the kernel is PE-bound.  Steady-state repeat-slope measured
36-50 us/iter across sessions (vs 86 us for the fp32r baseline; noise
between sessions is several us, so compare variants round-robin within one
process).  Chunk orderings "m" (m-outer, default) / "k0" / "k" differ by
less than the measurement noise.  Routing the steady adj/xcm loads through
SWDGE (gpsimd, default dma="gpsimd") instead of the sync HWDGE queue
reproducibly closed ~4-7 us of the full-vs-PE-only gap (paired A/B, two
rounds); routing the output stores through SWDGE as well (out_dma="gpsimd")
bought another ~4-6 us by the same mechanism.  All-DVE drains instead of
alternating DVE/Act measured neutral.
"""
import numpy as np

import concourse.bass as bass
import concourse.tile as tile
from concourse import bacc, mybir
from concourse.bass_utils import run_bass_kernel_spmd

# Problem shapes (hardcoded per the harness contract).
B, T, N, C = 8, 12, 2048, 64
TC = T * C                     # 768
P = 128                        # partitions
KT = N // P                    # 16 k tiles
NCHUNK = 512                   # node columns per chunk (one PSUM bank)
CHUNKS = N // NCHUNK           # 4
MT = TC // P                   # 6 tc (pair-of-timestep) tiles
NPAIR = T // 2                 # 6

F32 = mybir.dt.float32
F16 = mybir.dt.float16


def build_kernel(repeat=1, order="m", dma="gpsimd", out_dma="gpsimd",
                 drain="mix"):
    nc = bacc.Bacc(name="dgcn")

    # ---- per-core external inputs -------------------------------------
    xnode = nc.dram_tensor("xnode", [N, TC], F16, kind="ExternalInput")
    xcm = nc.dram_tensor("xcm", [TC, N], F16, kind="ExternalInput")
    adjt = nc.dram_tensor("adjt", [N, N], F16, kind="ExternalInput")
    sT = nc.dram_tensor("sT", [64, T], F32, kind="ExternalInput")        # st_emb.T
    w1 = nc.dram_tensor("w1", [64, 32], F32, kind="ExternalInput")
    b1p = nc.dram_tensor("b1p", [32, 1], F32, kind="ExternalInput")
    w2 = nc.dram_tensor("w2", [32, 1], F32, kind="ExternalInput")
    b2p = nc.dram_tensor("b2p", [1, 1], F32, kind="ExternalInput")
    # block-diagonal weight templates diag(Wk, Wk), [128, 128] each
    wd0 = nc.dram_tensor("wd0", [P, P], F32, kind="ExternalInput")
    wd1 = nc.dram_tensor("wd1", [P, P], F32, kind="ExternalInput")
    biasp = nc.dram_tensor("biasp", [P, 1], F32, kind="ExternalInput")   # bias twice
    masks = nc.dram_tensor("masks", [2, P], F32, kind="ExternalInput")   # upper/lower half sel
    out = nc.dram_tensor("out", [TC, N], F16, kind="ExternalOutput")

    out_ap = out.ap().rearrange("(m p) n -> p m n", p=P)
    xcm_ap = xcm.ap().rearrange("(m p) n -> p m n", p=P)
    adjt_ap = adjt.ap().rearrange("(k p) n -> p k n", p=P)
    xnode_ap = xnode.ap().rearrange("(k p) f -> p k f", p=P)

    with tile.TileContext(nc) as tc:
        with (
            tc.tile_pool(name="const", bufs=1) as const,
            tc.tile_pool(name="xn", bufs=1) as xn_pool,
            tc.tile_pool(name="adj", bufs=5) as adj_pool,
            tc.tile_pool(name="yts", bufs=13) as yts_pool,
            tc.tile_pool(name="xcmp", bufs=3) as xcm_pool,
            tc.tile_pool(name="outs", bufs=6) as outs_pool,
            tc.tile_pool(name="ytps", bufs=1, space="PSUM") as ytps_pool,
            tc.tile_pool(name="miscps", bufs=2, space="PSUM") as misc_ps,
        ):
            # ============ lambda MLP + paired scaled weights =============
            # tiny constants go through SWDGE (gpsimd) so they don't occupy
            # the serial HWDGE dispatch ring; loaded after the prologue's big
            # loads are issued (consts aren't needed until ~chunk-0 mid-loop).
            sT_sb = const.tile([64, T], F32)
            w1_sb = const.tile([64, 32], F32)
            b1_sb = const.tile([32, 1], F32)
            w2_sb = const.tile([32, 1], F32)
            b2_sb = const.tile([1, 1], F32)
            wd0_sb = const.tile([P, P], F32)
            wd1_sb = const.tile([P, P], F32)
            bias_sb = const.tile([P, 1], F32)
            mask_up = const.tile([1, P], F32)
            mask_lo = const.tile([1, P], F32)

            def load_consts():
                nc.gpsimd.dma_start(sT_sb[:], sT.ap())
                nc.gpsimd.dma_start(w1_sb[:], w1.ap())
                nc.gpsimd.dma_start(b1_sb[:], b1p.ap())
                nc.gpsimd.dma_start(w2_sb[:], w2.ap())
                nc.gpsimd.dma_start(b2_sb[:], b2p.ap())
                nc.gpsimd.dma_start(wd0_sb[:], wd0.ap())
                nc.gpsimd.dma_start(wd1_sb[:], wd1.ap())
                nc.gpsimd.dma_start(bias_sb[:], biasp.ap())
                nc.gpsimd.dma_start(mask_up[:], masks.ap()[0:1, :])
                nc.gpsimd.dma_start(mask_lo[:], masks.ap()[1:2, :])

            # scaled block-diagonal stationaries, fp16 (filled by mlp stages)
            wx_sb = const.tile([P, NPAIR, P], F16)   # identity-term weights
            wy_sb = const.tile([P, NPAIR, P], F16)   # adjacency-term weights
            hr_sb = const.tile([32, T], F32)
            lam_sb = const.tile([1, T], F32)
            cp_sb = const.tile([P, 2 * NPAIR], F32)

            # The lambda MLP's PE pieces are spread through chunk 0's k-loop
            # so each cross-engine hop (PE -> Act -> PE -> DVE -> PE) hides
            # under ~2.5us of adjacency matmuls instead of stalling the
            # in-order PE queue.
            def mlp_stage0():
                # h.T = relu(W1.T @ sT + b1)   [32, T]
                h_ps = misc_ps.tile([P, NCHUNK], F32, tag="mps", name="h_ps")
                nc.tensor.matmul(h_ps[:32, :T], w1_sb[:], sT_sb[:], start=True, stop=True)
                nc.scalar.activation(out=hr_sb[:], in_=h_ps[:32, :T],
                                     func=mybir.ActivationFunctionType.Relu,
                                     bias=b1_sb[:], scale=1.0)

            def mlp_stage1():
                # lam = 1 + relu(W2.T @ hr + b2), then c1/c2 coefficient prep
                lam_ps = misc_ps.tile([P, NCHUNK], F32, tag="mps", name="lam_ps")
                nc.tensor.matmul(lam_ps[:1, :T], w2_sb[:], hr_sb[:], start=True, stop=True)
                nc.scalar.activation(out=lam_sb[:], in_=lam_ps[:1, :T],
                                     func=mybir.ActivationFunctionType.Relu,
                                     bias=b2_sb[:], scale=1.0)
                lam1_sb = const.tile([1, T], F32)
                nc.vector.tensor_scalar_add(lam1_sb[:], lam_sb[:], 1.0)
                inv_sb = const.tile([1, T], F32)
                nc.vector.reciprocal(out=inv_sb[:], in_=lam1_sb[:])
                c2_sb = const.tile([1, T], F32)
                nc.vector.tensor_scalar_mul(c2_sb[:], inv_sb[:], 2.0)
                c1_sb = const.tile([1, T], F32)
                nc.vector.tensor_scalar(c1_sb[:], inv_sb[:], -2.0, 2.0,
                                        mybir.AluOpType.mult, mybir.AluOpType.add)
                mlp_stage1.c1 = c1_sb
                mlp_stage1.c2 = c2_sb

            def mlp_stage2():
                # paired per-partition coefficient columns:
                # cp[:, m] = [c1[2m]]*64 + [c1[2m+1]]*64, then the scaled
                # fp16 block-diagonal stationaries diag(c*Wk, c*Wk).
                c1_sb, c2_sb = mlp_stage1.c1, mlp_stage1.c2
                cp_ps = misc_ps.tile([P, NCHUNK], F32, tag="mps", name="cp_ps")
                c1_pairs = c1_sb.rearrange("p (a two) -> p two a", two=2)
                c2_pairs = c2_sb.rearrange("p (a two) -> p two a", two=2)
                nc.tensor.matmul(cp_ps[:, :NPAIR], mask_up[:], c1_pairs[:, 0, :],
                                 start=True, stop=False)
                nc.tensor.matmul(cp_ps[:, :NPAIR], mask_lo[:], c1_pairs[:, 1, :],
                                 start=False, stop=False)
                nc.tensor.matmul(cp_ps[:, NPAIR:2 * NPAIR], mask_up[:], c2_pairs[:, 0, :],
                                 start=False, stop=False)
                nc.tensor.matmul(cp_ps[:, NPAIR:2 * NPAIR], mask_lo[:], c2_pairs[:, 1, :],
                                 start=False, stop=True)
                nc.vector.tensor_copy(out=cp_sb[:], in_=cp_ps[:, :2 * NPAIR])
                for m in range(NPAIR):
                    nc.vector.tensor_scalar_mul(wx_sb[:, m, :], wd0_sb[:], cp_sb[:, m:m + 1])
                    nc.vector.tensor_scalar_mul(wy_sb[:, m, :], wd1_sb[:],
                                                cp_sb[:, NPAIR + m:NPAIR + m + 1])

            # ============ main loop with prefetch pipelining =============
            chunk_seq = [c for _ in range(repeat) for c in range(CHUNKS)]
            KB = 4   # k-tiles per batched (prefetched) DMA

            steady_dma = nc.gpsimd if dma == "gpsimd" else nc.sync

            def load_at(ch, b):
                # one DMA covering k-tiles 4b..4b+3 of this chunk's columns
                at_sb = adj_pool.tile([P, KB, NCHUNK], F16, tag="at", name="at_sb")
                cs = slice(ch * NCHUNK, (ch + 1) * NCHUNK)
                steady_dma.dma_start(at_sb[:], adjt_ap[:, KB * b:KB * (b + 1), cs])
                return at_sb

            def load_at1(ch, k):
                # single k-tile load (fine-grained, for the first chunk)
                at_sb = adj_pool.tile([P, NCHUNK], F16, tag="at1", name="at1_sb", bufs=16)
                cs = slice(ch * NCHUNK, (ch + 1) * NCHUNK)
                nc.sync.dma_start(at_sb[:], adjt.ap()[k * P:(k + 1) * P, cs])
                return at_sb

            def load_xcm(ch):
                # channel-major x slab for this chunk's columns (all 6 m)
                xc_sb = xcm_pool.tile([P, MT, NCHUNK], F16, tag="xcm", name="xcm_sb")
                cs = slice(ch * NCHUNK, (ch + 1) * NCHUNK)
                steady_dma.dma_start(xc_sb[:], xcm_ap[:, :, cs])
                return xc_sb

            def emit_final_m(ch, m, yts_list, xcm_sb, tail=False):
                cs = slice(ch * NCHUNK, (ch + 1) * NCHUNK)
                if tail:
                    # last chunk: the yt accumulator banks are free by now --
                    # use them so all six finals ping-pong without bank waits
                    o_ps = ytps_pool.tile([P, NCHUNK], F32, tag=f"yt{m}",
                                          name="o_ps_t")
                else:
                    o_ps = misc_ps.tile([P, NCHUNK], F32, tag="mps", name="o_ps")
                nc.tensor.matmul(o_ps[:], wx_sb[:, m, :], xcm_sb[:, m, :],
                                 start=True, stop=False)
                nc.tensor.matmul(o_ps[:], wy_sb[:, m, :], yts_list[m][:],
                                 start=False, stop=True)
                # bias add while evacuating PSUM; stream out per-m
                out_sb = outs_pool.tile([P, NCHUNK], F16, tag="outsb")
                nc.scalar.activation(out=out_sb[:], in_=o_ps[:],
                                     func=mybir.ActivationFunctionType.Identity,
                                     bias=bias_sb[:], scale=1.0)
                if out_dma == "gpsimd":
                    nc.gpsimd.dma_start(out_ap[:, m, cs], out_sb[:])
                else:
                    nc.scalar.dma_start(out_ap[:, m, cs], out_sb[:])

            # prologue: fine-grained interleaved loads (one k-tile per DMA)
            # so the first matmul can start ~1us in; later chunks use batched
            # prefetched DMAs to keep HWDGE dispatch count low.
            xnt = []
            cur_at1 = []
            for k in range(KT):
                t_ = xn_pool.tile([P, TC], F16, tag=f"xnt{k}", name=f"xnt{k}")
                if k == 0:
                    # issue just the m=0 slice of the first tile, then the
                    # first adjacency tile, then the rest: the very first
                    # matmul needs only these small slices, so it starts
                    # earlier than with whole-tile ordering
                    nc.sync.dma_start(t_[:, :P], xnode_ap[:, k, :P])
                    xnt.append(t_)
                    a0 = adj_pool.tile([P, NCHUNK], F16, tag="at1", name="a0", bufs=16)
                    cs0 = slice(chunk_seq[0] * NCHUNK, chunk_seq[0] * NCHUNK + NCHUNK)
                    nc.sync.dma_start(a0[:, :NCHUNK // 2],
                                      adjt.ap()[k * P:(k + 1) * P, cs0][:, :NCHUNK // 2])
                    cur_at1.append(a0)
                    nc.sync.dma_start(a0[:, NCHUNK // 2:],
                                      adjt.ap()[k * P:(k + 1) * P, cs0][:, NCHUNK // 2:])
                    nc.sync.dma_start(t_[:, P:], xnode_ap[:, k, P:])
                else:
                    nc.sync.dma_start(t_[:], xnode_ap[:, k, :])
                    xnt.append(t_)
                    cur_at1.append(load_at1(chunk_seq[0], k))
            load_consts()

            def xnt_slice(k, m):
                return xnt[k][:, m * P:(m + 1) * P]

            def drain_yt(yt_ps_m, m):
                yts_sb = yts_pool.tile([P, NCHUNK], F16, tag="yts", name="yts_sb")
                if drain == "dve" or m % 2 == 0:
                    nc.vector.tensor_copy(out=yts_sb[:], in_=yt_ps_m[:])
                else:
                    nc.scalar.activation(out=yts_sb[:], in_=yt_ps_m[:],
                                         func=mybir.ActivationFunctionType.Identity,
                                         scale=1.0)
                return yts_sb

            cur_xcm = None
            cur_at = None    # batched tiles for chunks after the first
            pending = None   # (ch, yts_list, xcm_sb) finals woven into next chunk
            for ci, ch in enumerate(chunk_seq):
                nxt = chunk_seq[ci + 1] if ci + 1 < len(chunk_seq) else None
                nxt_at = []

                yt_ps = [ytps_pool.tile([P, NCHUNK], F32, tag=f"yt{m}", name=f"yt{m}")
                         for m in range(MT)]
                yts_list = []
                if ci == 0:
                    # First chunk: k-outer so the PE consumes adjacency tiles
                    # in DMA arrival order; the lambda-MLP's cross-engine
                    # stages hide under the k-loop.
                    for k in range(KT - 1):
                        if nxt is not None and k % KB == 0:
                            nxt_at.append(load_at(nxt, k // KB))
                        if k == 2:
                            cur_xcm = load_xcm(ch)
                        rhs = cur_at1[k][:]
                        for m in range(MT):
                            if k == 0 and m == 0:
                                H = NCHUNK // 2
                                nc.tensor.matmul(yt_ps[0][:, :H], xnt_slice(0, 0),
                                                 cur_at1[0][:, :H], start=True, stop=False)
                                nc.tensor.matmul(yt_ps[0][:, H:], xnt_slice(0, 0),
                                                 cur_at1[0][:, H:], start=False, stop=False)
                                continue
                            nc.tensor.matmul(
                                yt_ps[m][:], xnt_slice(k, m), rhs,
                                start=(k == 0), stop=False,
                            )
                        if k == 4:
                            mlp_stage0()
                        elif k == 8:
                            mlp_stage1()
                        elif k == 12:
                            mlp_stage2()
                    # final k-tile interleaved with the PSUM evacuations
                    k = KT - 1
                    for m in range(MT):
                        nc.tensor.matmul(yt_ps[m][:], xnt_slice(k, m),
                                         cur_at1[k][:], start=False, stop=True)
                        yts_list.append(drain_yt(yt_ps[m], m))
                elif order == "m":
                    # Steady chunks: m-outer.  Each PSUM bank accumulates its
                    # 16 k-tiles back-to-back, then drains during the next
                    # bank's ~full m-pass -- no drain race at chunk seams.
                    # Previous chunk's finals + next chunk's prefetch DMAs
                    # are woven between m-passes.
                    for m in range(MT):
                        for k in range(KT):
                            nc.tensor.matmul(
                                yt_ps[m][:], xnt_slice(k, m),
                                cur_at[k // KB][:, k % KB, :],
                                start=(k == 0), stop=(k == KT - 1),
                            )
                        yts_list.append(drain_yt(yt_ps[m], m))
                        if nxt is not None and m < KT // KB:
                            nxt_at.append(load_at(nxt, m))
                        if m == 2:
                            cur_xcm = load_xcm(ch)
                        if pending is not None:
                            emit_final_m(pending[0], m, pending[1], pending[2])
                else:
                    # Steady chunks, k-outer: banks accumulate round-robin;
                    # the final k-tile is interleaved with the drains.
                    # order "k0" weaves the previous chunk's finals between
                    # the k=0 matmuls, right where the next chunk's
                    # start=True writes race the previous drains; order "k"
                    # spreads them at k>=4.
                    for k in range(KT - 1):
                        if nxt is not None and k % KB == 0:
                            nxt_at.append(load_at(nxt, k // KB))
                        if k == 2:
                            cur_xcm = load_xcm(ch)
                        rhs = cur_at[k // KB][:, k % KB, :]
                        for m in range(MT):
                            nc.tensor.matmul(
                                yt_ps[m][:], xnt_slice(k, m), rhs,
                                start=(k == 0), stop=False,
                            )
                            if (order == "k0" and k == 0
                                    and pending is not None):
                                emit_final_m(pending[0], m,
                                             pending[1], pending[2])
                        if (order == "k" and pending is not None
                                and k >= 4 and (k - 4) % 2 == 0):
                            emit_final_m(pending[0], (k - 4) // 2,
                                         pending[1], pending[2])
                    k = KT - 1
                    rhs = cur_at[k // KB][:, k % KB, :]
                    for m in range(MT):
                        nc.tensor.matmul(yt_ps[m][:], xnt_slice(k, m), rhs,
                                         start=False, stop=True)
                        yts_list.append(drain_yt(yt_ps[m], m))

                pending = (ch, yts_list, cur_xcm)
                cur_at = nxt_at
            # tail: last chunk's finals
            for m in range(MT):
                emit_final_m(pending[0], m, pending[1], pending[2], tail=True)

    nc.finalize()
    return nc


_NC_CACHE = None


def _get_nc():
    global _NC_CACHE
    if _NC_CACHE is None:
        _NC_CACHE = build_kernel()
    return _NC_CACHE


def prep_in_maps(x, adj, st_emb, weights, bias, W1, b1, W2, b2):
    """Host-side layout prep -> per-core input dicts."""
    x = np.asarray(x, dtype=np.float32)
    adj = np.asarray(adj, dtype=np.float32)
    st_emb = np.asarray(st_emb, dtype=np.float32)
    weights = np.asarray(weights, dtype=np.float32)
    bias = np.asarray(bias, dtype=np.float32)
    W1 = np.asarray(W1, dtype=np.float32)
    b1 = np.asarray(b1, dtype=np.float32)
    W2 = np.asarray(W2, dtype=np.float32)
    b2 = np.asarray(b2, dtype=np.float32)

    adjT = np.ascontiguousarray(adj.T.astype(np.float16))
    sT = np.ascontiguousarray(st_emb.reshape(T, 64).T)          # [64, T]
    w0g, w1g = weights[0], weights[1]                            # [64, 64] each
    z = np.zeros((64, 64), np.float32)
    wd0 = np.block([[w0g, z], [z, w0g]])                         # [128, 128]
    wd1 = np.block([[w1g, z], [z, w1g]])
    biasp = np.concatenate([bias, bias]).reshape(P, 1)
    masks = np.zeros((2, P), np.float32)
    masks[0, :64] = 1.0
    masks[1, 64:] = 1.0
    b1p = b1.reshape(32, 1)
    b2p = b2.reshape(1, 1)

    shared = {
        "adjt": adjT, "sT": sT, "w1": W1, "b1p": b1p, "w2": W2, "b2p": b2p,
        "wd0": wd0, "wd1": wd1, "biasp": biasp, "masks": masks,
    }
    in_maps = []
    for b in range(B):
        xb = x[b].astype(np.float16)                             # [T, N, C]
        xnode = np.ascontiguousarray(xb.transpose(1, 0, 2).reshape(N, TC))
        xcm = np.ascontiguousarray(xb.transpose(0, 2, 1).reshape(TC, N))
        in_maps.append({"xnode": xnode, "xcm": xcm, **shared})
    return in_maps


def assemble_output(results):
    """Per-core [TC, N] f16 -> full [B, T, N, C] f32."""
    outs = []
    for r in results:
        oc = r["out"].astype(np.float32).reshape(T, 64, N).transpose(0, 2, 1)
        outs.append(oc)
    return np.stack(outs, axis=0)


def run(inputs, **spmd_kwargs):
    nc = _get_nc()
    in_maps = prep_in_maps(**inputs)
    res = run_bass_kernel_spmd(nc, in_maps, core_ids=list(range(B)), **spmd_kwargs)
    return assemble_output(res.results), res


def kernel(**inputs) -> np.ndarray:
    out, _ = run(inputs)
    return out
